# revision 18
# baseline (speedup 1.0000x reference)
"""Trainium2 Bass kernel for Exercise-KC GraphConvolution (concat=True branch).

Computes: elu((adj @ (kc_h @ W1)) * (ex_h @ W1 @ W2))   -> [50000, 512]

Strategy (8 NeuronCores):
  - Shard exercise rows across cores: pad 50000 -> 51200 = 8 * 50 * 128.
  - Replicate kc_h / W1 / W2; fold ex@W1@W2 into ex@(W1@W2) (one matmul).
  - Host-side layout prep supplies all matmul stationary operands pre-transposed
    (k-major, partition = k % 128) so the device does only contiguous DMA loads.
  - Supertiles of 2 row-tiles: 2MB adj DMAs (16KB/partition packets) on the
    sync HWDGE ring; exercise loads + output stores on the scalar ring.
  - Per 128-row output tile: 16 accumulating matmuls for the spmm (K=2048)
    + 4 for the exercise branch (K=512), both N=512, then
    elu(x) = max(x, exp(min(x,0)) - 1) on vector+scalar engines.
  - Matmuls in float32r (1 col/cycle vs 4 for fp32); operands pre-rounded
    on host to the fp32r grid (RNE to 11-bit mantissa).
"""

import os

import numpy as np

import concourse.bass as bass
import concourse.mybir as mybir
import concourse.tile as tile
from concourse import bacc
from concourse.bass_utils import run_bass_kernel_spmd

N_EX = 50000
IN_F = 512
OUT_F = 512
N_KC = 2048
N_CORES = 8

P = 128                       # partitions
S = 2                         # row-tiles per supertile
N_SUPER = 25                  # supertiles per core
T_PER_CORE = S * N_SUPER      # 50 row-tiles per core
E_PER_CORE = T_PER_CORE * P   # 6400
E_PAD = N_CORES * E_PER_CORE  # 51200
KHI_ADJ = N_KC // P           # 16
KHI_IN = IN_F // P            # 4
FB = OUT_F                    # 512 (psum free dim)
KHI_TOT = KHI_ADJ + KHI_IN    # 20 combined k-chunks per row-tile

F32 = mybir.dt.float32
# float32r: 20-bit float (8e11m, low 12 bits zero) through the PE at full rate
# (1 col/cycle vs 4 for fp32). Operands must be pre-rounded; we round on host.
_MM_DT_ENV = os.environ.get("KERNEL_MM_DT", "f32r")
MM_DT = mybir.dt.float32r if _MM_DT_ENV == "f32r" else mybir.dt.float32


def round_fp32r(a: np.ndarray) -> np.ndarray:
    """Round fp32 -> fp32r bit pattern (RNE to 11-bit mantissa) on host."""
    if MM_DT != mybir.dt.float32r:
        return np.ascontiguousarray(a, dtype=np.float32)
    u = np.ascontiguousarray(a, dtype=np.float32).view(np.uint32)
    lsb = (u >> np.uint32(12)) & np.uint32(1)
    ur = (u + np.uint32(0x7FF) + lsb) & np.uint32(0xFFFFF000)
    return ur.view(np.float32)


def build_nc(n_super: int = N_SUPER, mm_dt=None):
    """Build + compile the per-core Bass program (same program on all cores)."""
    if mm_dt is None:
        mm_dt = MM_DT
    nc = bacc.Bacc(
        "TRN2",
        target_bir_lowering=False,
        debug=False,
        enable_asserts=False,
        num_devices=N_CORES,
    )
    AF = mybir.ActivationFunctionType
    OP = mybir.AluOpType
    MD = mm_dt

    # combined stream: 16 adj k-chunks + 4 exercise k-chunks per row-tile
    comb = nc.dram_tensor("comb", [n_super, P, S, KHI_TOT, P], MD,
                          kind="ExternalInput")
    # kc_h^T in 4 chunks of 512 kc-rows so setup compute pipelines with DMA
    kct = nc.dram_tensor("kct", [KHI_IN, P, KHI_IN, FB], MD,
                         kind="ExternalInput")
    w1r = nc.dram_tensor("w1r", [P, KHI_IN, FB], MD, kind="ExternalInput")
    w1t = nc.dram_tensor("w1t", [P, KHI_IN, FB], MD, kind="ExternalInput")
    w2r = nc.dram_tensor("w2r", [P, KHI_IN, FB], MD, kind="ExternalInput")
    outp = nc.dram_tensor("outp", [n_super, P, S, FB], F32,
                          kind="ExternalOutput")

    with tile.TileContext(nc) as tc:
        with (
            tc.tile_pool(name="const", bufs=1) as constp,
            tc.tile_pool(name="adj", bufs=3) as adjp,
            tc.tile_pool(name="outb", bufs=3) as outbp,
            tc.tile_pool(name="tmp", bufs=3) as tmpp,
            tc.tile_pool(name="ps", bufs=2, space=bass.MemorySpace.PSUM) as psp,
            tc.tile_pool(name="psq", bufs=2, space=bass.MemorySpace.PSUM) as psq,
        ):
            # ---- setup: load constants (small weights first on both rings,
            #      kc_h^T chunks split across rings so PE unblocks early) ----
            w1t_sb = constp.tile([P, KHI_IN, FB], MD)
            nc.scalar.dma_start(w1t_sb[:], w1t[:])
            w2r_sb = constp.tile([P, KHI_IN, FB], MD)
            nc.sync.dma_start(w2r_sb[:], w2r[:])
            w1r_sb = constp.tile([P, KHI_IN, FB], MD)
            nc.sync.dma_start(w1r_sb[:], w1r[:])
            kct_sbs = []
            for g in range(KHI_IN):
                kg = constp.tile([P, KHI_IN, FB], MD, tag=f"kct{g}")
                eng = nc.sync if g % 2 == 0 else nc.scalar
                eng.dma_start(kg[:], kct[g])
                kct_sbs.append(kg)

            # ---- setup: W12 = W1 @ W2  ([512, 512], stored k-major) ----
            w12_sb = constp.tile([P, KHI_IN, FB], MD)
            for m_t in range(KHI_IN):
                pw = psq.tile([P, FB], F32)
                for k_hi in range(KHI_IN):
                    nc.tensor.matmul(
                        pw[:],
                        w1t_sb[:, k_hi, bass.ts(m_t, P)],
                        w2r_sb[:, k_hi, :],
                        start=(k_hi == 0),
                        stop=(k_hi == KHI_IN - 1),
                    )
                nc.vector.tensor_copy(w12_sb[:, m_t, :], pw[:])

            # ---- setup: kcWh = kc_h @ W1  ([2048, 512], stored k-major) ----
            kcwh_sb = constp.tile([P, KHI_ADJ, FB], MD)
            for g in range(KHI_IN):
                for mt_local in range(KHI_IN):
                    m_t = g * KHI_IN + mt_local
                    pk = psq.tile([P, FB], F32)
                    for k_hi in range(KHI_IN):
                        nc.tensor.matmul(
                            pk[:],
                            kct_sbs[g][:, k_hi, bass.ts(mt_local, P)],
                            w1r_sb[:, k_hi, :],
                            start=(k_hi == 0),
                            stop=(k_hi == KHI_IN - 1),
                        )
                    nc.vector.tensor_copy(kcwh_sb[:, m_t, :], pk[:])

            # ---- main loop over supertiles (S row-tiles each) ----
            for su in range(n_super):
                a_sb = adjp.tile([P, S, KHI_TOT, P], MD)
                nc.sync.dma_start(a_sb[:], comb[su])
                o_sb = outbp.tile([P, S, FB], F32)

                for ti in range(S):
                    ps_s = psp.tile([P, FB], F32)  # spmm branch
                    for k_hi in range(KHI_ADJ):
                        nc.tensor.matmul(
                            ps_s[:],
                            a_sb[:, ti, k_hi, :],
                            kcwh_sb[:, k_hi, :],
                            start=(k_hi == 0),
                            stop=(k_hi == KHI_ADJ - 1),
                        )
                    ps_e = psp.tile([P, FB], F32)  # exercise branch
                    for k_hi in range(KHI_IN):
                        nc.tensor.matmul(
                            ps_e[:],
                            a_sb[:, ti, KHI_ADJ + k_hi, :],
                            w12_sb[:, k_hi, :],
                            start=(k_hi == 0),
                            stop=(k_hi == KHI_IN - 1),
                        )

                    # elu(prod) = max(prod, exp(min(prod,0)) - 1)
                    exb = tmpp.tile([P, FB], F32)
                    nc.scalar.copy(exb[:], ps_e[:])  # one PSUM operand max
                    prod = tmpp.tile([P, FB], F32)
                    nc.vector.tensor_tensor(prod[:], ps_s[:], exb[:], OP.mult)
                    nmin = tmpp.tile([P, FB], F32)
                    nc.vector.tensor_scalar(nmin[:], prod[:], 0.0, None, OP.min)
                    expv = tmpp.tile([P, FB], F32)
                    nc.scalar.activation(expv[:], nmin[:], AF.Exp)
                    nc.vector.scalar_tensor_tensor(
                        o_sb[:, ti, :], expv[:], -1.0, prod[:], OP.add, OP.max)

                nc.scalar.dma_start(outp[su], o_sb[:])

    nc.compile()
    return nc


def prep_inputs(exercise_h, kc_h, adj_exercise_kc, W1, W2,
                n_super: int = N_SUPER):
    """Host-side shard + layout prep. Returns in_maps (one dict per core)."""
    ex = round_fp32r(exercise_h)
    kc = round_fp32r(kc_h)
    adj = round_fp32r(adj_exercise_kc)
    w1 = round_fp32r(W1)
    w2 = round_fp32r(W2)

    e_pad = N_CORES * n_super * S * P
    n_rows = min(N_EX, e_pad)

    adj_p = np.zeros((e_pad, N_KC), np.float32)
    adj_p[:n_rows] = adj[:n_rows]
    ex_p = np.zeros((e_pad, IN_F), np.float32)
    ex_p[:n_rows] = ex[:n_rows]
    # combined [core, su, k_lo, ti, k_hi_tot, e]: adj chunks then ex chunks
    comb = np.empty((N_CORES, n_super, P, S, KHI_TOT, P), np.float32)
    comb[:, :, :, :, :KHI_ADJ, :] = (
        adj_p.reshape(N_CORES, n_super, S, P, KHI_ADJ, P)
        .transpose(0, 1, 5, 2, 4, 3))
    comb[:, :, :, :, KHI_ADJ:, :] = (
        ex_p.reshape(N_CORES, n_super, S, P, KHI_IN, P)
        .transpose(0, 1, 5, 2, 4, 3))

    # [g, k_lo, k_hi, m'] with m = g*512 + m'
    kct = np.ascontiguousarray(
        kc.reshape(KHI_IN, FB, KHI_IN, P).transpose(0, 3, 2, 1))
    w1r = np.ascontiguousarray(w1.reshape(KHI_IN, P, FB).transpose(1, 0, 2))
    w1t = np.ascontiguousarray(w1.reshape(IN_F, KHI_IN, P).transpose(2, 1, 0))
    w2r = np.ascontiguousarray(w2.reshape(KHI_IN, P, FB).transpose(1, 0, 2))

    return [
        {"comb": comb[c], "kct": kct,
         "w1r": w1r, "w1t": w1t, "w2r": w2r}
        for c in range(N_CORES)
    ]


def unpack_output(results, n_super: int = N_SUPER) -> np.ndarray:
    """results: list per core of {\"outp\": [n_super, P, S, FB]} -> [N_EX, FB]."""
    per_core = [
        np.asarray(r["outp"]).transpose(0, 2, 1, 3).reshape(n_super * S * P, FB)
        for r in results
    ]
    return np.concatenate(per_core, axis=0)[:N_EX]


_NC_CACHE: dict = {}


def _get_nc():
    key = (N_SUPER, MM_DT)
    if key not in _NC_CACHE:
        _NC_CACHE[key] = build_nc()
    return _NC_CACHE[key]


def kernel(exercise_h, kc_h, adj_exercise_kc, W1, W2):
    nc = _get_nc()
    in_maps = prep_inputs(exercise_h, kc_h, adj_exercise_kc, W1, W2)
    res = run_bass_kernel_spmd(nc, in_maps, core_ids=list(range(N_CORES)))
    return np.ascontiguousarray(unpack_output(res.results))


# revision 19
# speedup vs baseline: 1.0389x; 1.0389x over previous
"""Trainium2 Bass kernel for Exercise-KC GraphConvolution (concat=True branch).

Computes: elu((adj @ (kc_h @ W1)) * (ex_h @ W1 @ W2))   -> [50000, 512]

Strategy (8 NeuronCores):
  - Shard exercise rows across cores: pad 50000 -> 51200 = 8 * 50 * 128.
  - Replicate kc_h / W1 / W2; fold ex@W1@W2 into ex@(W1@W2) (one matmul).
  - Host-side layout prep supplies all matmul stationary operands pre-transposed
    (k-major, partition = k % 128) so the device does only contiguous DMA loads.
  - Supertiles of 2 row-tiles; adjacency + exercise data fused into one
    combined stream (one 2.5MB DMA per supertile, 20KB/partition packets) on
    the sync HWDGE ring; weight/kc loads split across both rings; output
    stores on the scalar ring.
  - Per 128-row output tile: 16 accumulating matmuls for the spmm (K=2048)
    + 4 for the exercise branch (K=512), both N=512, then
    elu(x) = max(x, exp(min(x,0)) - 1) on vector+scalar engines.
  - Matmuls in float32r (1 col/cycle vs 4 for fp32); operands pre-rounded
    on host to the fp32r grid (RNE to 11-bit mantissa).
"""

import os

import numpy as np

import concourse.bass as bass
import concourse.mybir as mybir
import concourse.tile as tile
from concourse import bacc
from concourse.bass_utils import run_bass_kernel_spmd

N_EX = 50000
IN_F = 512
OUT_F = 512
N_KC = 2048
N_CORES = 8

P = 128                       # partitions
S = 2                         # row-tiles per supertile
N_SUPER = 25                  # supertiles per core
T_PER_CORE = S * N_SUPER      # 50 row-tiles per core
E_PER_CORE = T_PER_CORE * P   # 6400
E_PAD = N_CORES * E_PER_CORE  # 51200
KHI_ADJ = N_KC // P           # 16
KHI_IN = IN_F // P            # 4
FB = OUT_F                    # 512 (psum free dim)
KHI_TOT = KHI_ADJ + KHI_IN    # 20 combined k-chunks per row-tile

F32 = mybir.dt.float32
# float32r: 20-bit float (8e11m, low 12 bits zero) through the PE at full rate
# (1 col/cycle vs 4 for fp32). Operands must be pre-rounded; we round on host.
_MM_DT_ENV = os.environ.get("KERNEL_MM_DT", "f32r")
MM_DT = mybir.dt.float32r if _MM_DT_ENV == "f32r" else mybir.dt.float32


def round_fp32r(a: np.ndarray) -> np.ndarray:
    """Round fp32 -> fp32r bit pattern (RNE to 11-bit mantissa) on host."""
    if MM_DT != mybir.dt.float32r:
        return np.ascontiguousarray(a, dtype=np.float32)
    u = np.ascontiguousarray(a, dtype=np.float32).view(np.uint32)
    lsb = (u >> np.uint32(12)) & np.uint32(1)
    ur = (u + np.uint32(0x7FF) + lsb) & np.uint32(0xFFFFF000)
    return ur.view(np.float32)


def build_nc(n_super: int = N_SUPER, mm_dt=None):
    """Build + compile the per-core Bass program (same program on all cores)."""
    if mm_dt is None:
        mm_dt = MM_DT
    nc = bacc.Bacc(
        "TRN2",
        target_bir_lowering=False,
        debug=False,
        enable_asserts=False,
        num_devices=N_CORES,
    )
    AF = mybir.ActivationFunctionType
    OP = mybir.AluOpType
    MD = mm_dt

    # combined stream: 16 adj k-chunks + 4 exercise k-chunks per row-tile
    comb = nc.dram_tensor("comb", [n_super, P, S, KHI_TOT, P], MD,
                          kind="ExternalInput")
    # kc_h^T in 4 chunks of 512 kc-rows so setup compute pipelines with DMA
    kct = nc.dram_tensor("kct", [KHI_IN, P, KHI_IN, FB], MD,
                         kind="ExternalInput")
    w1r = nc.dram_tensor("w1r", [P, KHI_IN, FB], MD, kind="ExternalInput")
    w1t = nc.dram_tensor("w1t", [P, KHI_IN, FB], MD, kind="ExternalInput")
    w2r = nc.dram_tensor("w2r", [P, KHI_IN, FB], MD, kind="ExternalInput")
    outp = nc.dram_tensor("outp", [n_super, P, S, FB], F32,
                          kind="ExternalOutput")

    with tile.TileContext(nc) as tc:
        with (
            tc.tile_pool(name="const", bufs=1) as constp,
            tc.tile_pool(name="adj", bufs=3) as adjp,
            tc.tile_pool(name="outb", bufs=3) as outbp,
            tc.tile_pool(name="tmp", bufs=3) as tmpp,
            tc.tile_pool(name="ps", bufs=2, space=bass.MemorySpace.PSUM) as psp,
            tc.tile_pool(name="psq", bufs=2, space=bass.MemorySpace.PSUM) as psq,
        ):
            # ---- setup: load constants (small weights first on both rings,
            #      kc_h^T chunks split across rings so PE unblocks early) ----
            w1t_sb = constp.tile([P, KHI_IN, FB], MD)
            nc.scalar.dma_start(w1t_sb[:], w1t[:])
            w2r_sb = constp.tile([P, KHI_IN, FB], MD)
            nc.sync.dma_start(w2r_sb[:], w2r[:])
            w1r_sb = constp.tile([P, KHI_IN, FB], MD)
            nc.sync.dma_start(w1r_sb[:], w1r[:])
            kct_sbs = []
            for g in range(KHI_IN):
                kg = constp.tile([P, KHI_IN, FB], MD, tag=f"kct{g}")
                eng = nc.sync if g % 2 == 0 else nc.scalar
                eng.dma_start(kg[:], kct[g])
                kct_sbs.append(kg)

            # ---- setup: W12 = W1 @ W2  ([512, 512], stored k-major) ----
            w12_sb = constp.tile([P, KHI_IN, FB], MD)
            for m_t in range(KHI_IN):
                pw = psq.tile([P, FB], F32)
                for k_hi in range(KHI_IN):
                    nc.tensor.matmul(
                        pw[:],
                        w1t_sb[:, k_hi, bass.ts(m_t, P)],
                        w2r_sb[:, k_hi, :],
                        start=(k_hi == 0),
                        stop=(k_hi == KHI_IN - 1),
                    )
                nc.vector.tensor_copy(w12_sb[:, m_t, :], pw[:])

            # ---- setup: kcWh = kc_h @ W1  ([2048, 512], stored k-major) ----
            kcwh_sb = constp.tile([P, KHI_ADJ, FB], MD)
            for g in range(KHI_IN):
                for mt_local in range(KHI_IN):
                    m_t = g * KHI_IN + mt_local
                    pk = psq.tile([P, FB], F32)
                    for k_hi in range(KHI_IN):
                        nc.tensor.matmul(
                            pk[:],
                            kct_sbs[g][:, k_hi, bass.ts(mt_local, P)],
                            w1r_sb[:, k_hi, :],
                            start=(k_hi == 0),
                            stop=(k_hi == KHI_IN - 1),
                        )
                    nc.vector.tensor_copy(kcwh_sb[:, m_t, :], pk[:])

            # ---- main loop over supertiles (S row-tiles each) ----
            for su in range(n_super):
                a_sb = adjp.tile([P, S, KHI_TOT, P], MD)
                nc.sync.dma_start(a_sb[:], comb[su])
                o_sb = outbp.tile([P, S, FB], F32)

                for ti in range(S):
                    ps_s = psp.tile([P, FB], F32)  # spmm branch
                    for k_hi in range(KHI_ADJ):
                        nc.tensor.matmul(
                            ps_s[:],
                            a_sb[:, ti, k_hi, :],
                            kcwh_sb[:, k_hi, :],
                            start=(k_hi == 0),
                            stop=(k_hi == KHI_ADJ - 1),
                        )
                    ps_e = psp.tile([P, FB], F32)  # exercise branch
                    for k_hi in range(KHI_IN):
                        nc.tensor.matmul(
                            ps_e[:],
                            a_sb[:, ti, KHI_ADJ + k_hi, :],
                            w12_sb[:, k_hi, :],
                            start=(k_hi == 0),
                            stop=(k_hi == KHI_IN - 1),
                        )

                    # elu(prod) = max(prod, exp(min(prod,0)) - 1)
                    exb = tmpp.tile([P, FB], F32)
                    nc.scalar.copy(exb[:], ps_e[:])  # one PSUM operand max
                    prod = tmpp.tile([P, FB], F32)
                    nc.vector.tensor_tensor(prod[:], ps_s[:], exb[:], OP.mult)
                    nmin = tmpp.tile([P, FB], F32)
                    nc.vector.tensor_scalar(nmin[:], prod[:], 0.0, None, OP.min)
                    expv = tmpp.tile([P, FB], F32)
                    nc.scalar.activation(expv[:], nmin[:], AF.Exp)
                    nc.vector.scalar_tensor_tensor(
                        o_sb[:, ti, :], expv[:], -1.0, prod[:], OP.add, OP.max)

                nc.scalar.dma_start(outp[su], o_sb[:])

    nc.compile()
    return nc


def prep_inputs(exercise_h, kc_h, adj_exercise_kc, W1, W2,
                n_super: int = N_SUPER):
    """Host-side shard + layout prep. Returns in_maps (one dict per core)."""
    ex = round_fp32r(exercise_h)
    kc = round_fp32r(kc_h)
    adj = round_fp32r(adj_exercise_kc)
    w1 = round_fp32r(W1)
    w2 = round_fp32r(W2)

    e_pad = N_CORES * n_super * S * P
    n_rows = min(N_EX, e_pad)

    adj_p = np.zeros((e_pad, N_KC), np.float32)
    adj_p[:n_rows] = adj[:n_rows]
    ex_p = np.zeros((e_pad, IN_F), np.float32)
    ex_p[:n_rows] = ex[:n_rows]
    # combined [core, su, k_lo, ti, k_hi_tot, e]: adj chunks then ex chunks
    comb = np.empty((N_CORES, n_super, P, S, KHI_TOT, P), np.float32)
    comb[:, :, :, :, :KHI_ADJ, :] = (
        adj_p.reshape(N_CORES, n_super, S, P, KHI_ADJ, P)
        .transpose(0, 1, 5, 2, 4, 3))
    comb[:, :, :, :, KHI_ADJ:, :] = (
        ex_p.reshape(N_CORES, n_super, S, P, KHI_IN, P)
        .transpose(0, 1, 5, 2, 4, 3))

    # [g, k_lo, k_hi, m'] with m = g*512 + m'
    kct = np.ascontiguousarray(
        kc.reshape(KHI_IN, FB, KHI_IN, P).transpose(0, 3, 2, 1))
    w1r = np.ascontiguousarray(w1.reshape(KHI_IN, P, FB).transpose(1, 0, 2))
    w1t = np.ascontiguousarray(w1.reshape(IN_F, KHI_IN, P).transpose(2, 1, 0))
    w2r = np.ascontiguousarray(w2.reshape(KHI_IN, P, FB).transpose(1, 0, 2))

    return [
        {"comb": comb[c], "kct": kct,
         "w1r": w1r, "w1t": w1t, "w2r": w2r}
        for c in range(N_CORES)
    ]


def unpack_output(results, n_super: int = N_SUPER) -> np.ndarray:
    """results: list per core of {\"outp\": [n_super, P, S, FB]} -> [N_EX, FB]."""
    per_core = [
        np.asarray(r["outp"]).transpose(0, 2, 1, 3).reshape(n_super * S * P, FB)
        for r in results
    ]
    return np.concatenate(per_core, axis=0)[:N_EX]


_NC_CACHE: dict = {}


def _get_nc():
    key = (N_SUPER, MM_DT)
    if key not in _NC_CACHE:
        _NC_CACHE[key] = build_nc()
    return _NC_CACHE[key]


def kernel(exercise_h, kc_h, adj_exercise_kc, W1, W2):
    nc = _get_nc()
    in_maps = prep_inputs(exercise_h, kc_h, adj_exercise_kc, W1, W2)
    res = run_bass_kernel_spmd(nc, in_maps, core_ids=list(range(N_CORES)))
    return np.ascontiguousarray(unpack_output(res.results))
